# revision 15
# baseline (speedup 1.0000x reference)
"""BuddingLayer Trainium2 kernel (8-core, routed + fp8, contiguous-stream).

Reference computation (N = size_in = 8192, O = size_out = 8192):
    mask  = (x * saturated) != 0                   # ~half the neurons
    h2    = tiny per-neuron MLP(x)                              [N,3]
    h3    = relu(sum_i W3[n,o,i] * h2[n,i] + b3[n,o])           [N,O]
    u[o]  = sum_n mask[n] * h3[n,o]
    out   = weight @ (x * ~saturated) + bias + u

Host-side routing gathers the ~4112 active experts (mask=1) and ~4080
active dense columns; all big streams are fp8e4 (tolerance 2e-2).
Per-core stream ~21 MB -> ~60 us DMA floor at 358 GB/s.

v2 design (from 113.5us baseline profile):
  * The baseline spent ~130us of aggregate issue-engine time on ~200
    small strided DMAs (512B packets) and ~25us of Tensor time on bf16
    ones-reduce matmuls.  Both are restructured away:
  * Expert stream is pre-tiled on host into per-o-block CONTIGUOUS
    super-tiles pe[b] = [128, ns, 2(pair), 2(row), 512] fp8 -> ONE
    1 MB DMA per block (16 total), 8 KB/partition contiguous.
  * All ns=4 slabs go through the PE DoubleRow path (2 matmuls/slab
    into stk_s).  relu is a 2-op chain per psum tile pair: Scalar
    ACTIVATE(relu) writes row r of a [128,2,512] fp8 acc tile for slab
    2r, DVE scalar_tensor_tensor max-add accumulates slab 2r+1.
  * One DoubleRow reduce matmul per block contracts acc against a
    0.125-valued block-column selector -> row b of a single [16,512]
    psum bank accumulated across all 16 blocks; one copy + one store
    at the end.  (0.125 = 1/H2S undoes the stationary's 8x scale.)
  * Dense matvec: wt pre-tiled into 8 contiguous pair-tiles
    [128, 2(kc), 2(row), 1024] fp8, DoubleRow with xg stationary.
  * Small MLP consts for the 4 slab groups + 2 tail experts are merged
    (nt=6) into single per-field DMAs.
"""

import sys

import numpy as np

_TRN = "/opt/trn_rl_repo"
if _TRN not in sys.path:
    sys.path.insert(0, _TRN)

import ml_dtypes

import concourse.bacc as bacc
import concourse.mybir as mybir
from concourse import tile
from concourse.bass_utils import run_bass_kernel_spmd

F32 = mybir.dt.float32
BF16 = mybir.dt.bfloat16
FP8 = mybir.dt.float8e4
AF = mybir.ActivationFunctionType
ALU = mybir.AluOpType
AX = mybir.AxisListType
PM = mybir.MatmulPerfMode

NP_BF16 = ml_dtypes.bfloat16
NP_FP8 = ml_dtypes.float8_e4m3

N_CORES = 8
SIZE_IN = 8192
SIZE_OUT = 8192
OC = SIZE_OUT // 128          # o-chunks for the tail layout
O_BLK = 512                   # o-block (one psum bank of f32)
WT_SCALE = 1024.0             # dense weights are subnormal in fp8; prescale
H2S = 8.0                     # h2 scale in the PE stationary (undone in reduce)


def build_program(
    ns,                        # 128-expert PE slabs per core (must be even)
    n_tail,                    # leftover experts per core (o-transposed path)
    nkc2,                      # 256-row DoubleRow chunks for dense (even)
    size_out=SIZE_OUT,
    n_cores=N_CORES,
    pe_bufs=16,
    wt_bufs=8,
    acc_bufs=3,
    tail_blk=10,
    enable_asserts=False,
):
    assert ns % 2 == 0 and ns >= 2
    assert nkc2 % 2 == 0
    m_own = size_out // n_cores
    NB = size_out // O_BLK
    npair = nkc2 // 2
    nrow = ns // 2
    nt_all = ns + n_tail       # merged small-MLP groups (slab + tail)

    nc = bacc.Bacc(
        "TRN2",
        target_bir_lowering=False,
        debug=False,
        enable_asserts=enable_asserts,
        num_devices=n_cores,
    )

    d = {}
    d["pe"] = nc.dram_tensor("pe", [NB, 128, ns, 2, 2, O_BLK], FP8,
                             kind="ExternalInput")
    d["wt"] = nc.dram_tensor("wt", [npair, 128, 2, 2, m_own], FP8,
                             kind="ExternalInput")
    # packed consts: fpk = cind | xg (fp8), cpk = x | w1 | b1 | w2 | b2 (f32)
    d["fpk"] = nc.dram_tensor("fpk", [128, 128 + 2 * nkc2], FP8,
                              kind="ExternalInput")
    d["cpk"] = nc.dram_tensor("cpk", [128, 25 * nt_all], F32,
                              kind="ExternalInput")
    d["bias"] = nc.dram_tensor("bias", [1, m_own], F32, kind="ExternalInput")
    if n_tail:
        d["tbf"] = nc.dram_tensor("tbf", [128, n_tail, 4, OC], BF16,
                                  kind="ExternalInput")
        d["ut_out"] = nc.dram_tensor("ut_out", [128, OC], F32,
                                     kind="ExternalOutput")
    d["u_out"] = nc.dram_tensor("u_out", [NB, O_BLK], F32, kind="ExternalOutput")
    d["dense_out"] = nc.dram_tensor("dense_out", [1, m_own], F32,
                                    kind="ExternalOutput")

    def small_mlp(cp, x_sb, w1, b1, w2, b2, nt, pfx):
        h0 = cp.tile([128, nt], F32, tag=f"{pfx}h0")
        nc.vector.tensor_scalar_mul(h0[:], x_sb[:], 1.0 / 3.0)
        rs1 = cp.tile([128, nt, 3], F32, tag=f"{pfx}rs1")
        nc.vector.tensor_reduce(rs1[:], w1[:], axis=AX.X, op=ALU.add)
        h1 = cp.tile([128, nt, 3], F32, tag=f"{pfx}h1")
        for t in range(nt):
            nc.vector.scalar_tensor_tensor(
                h1[:, t, :], rs1[:, t, :], h0[:, t : t + 1], b1[:, t, :],
                op0=ALU.mult, op1=ALU.add,
            )
        nc.vector.tensor_scalar_max(h1[:], h1[:], 0.0)
        h2 = cp.tile([128, nt, 3], F32, tag=f"{pfx}h2")
        for t in range(nt):
            nc.vector.scalar_tensor_tensor(
                h2[:, t, :], w2[:, t, :, 0], h1[:, t, 0:1], b2[:, t, :],
                op0=ALU.mult, op1=ALU.add,
            )
            for i in (1, 2):
                nc.vector.scalar_tensor_tensor(
                    h2[:, t, :], w2[:, t, :, i], h1[:, t, i : i + 1], h2[:, t, :],
                    op0=ALU.mult, op1=ALU.add,
                )
        nc.vector.tensor_scalar_max(h2[:], h2[:], 0.0)
        return h2

    with tile.TileContext(nc) as tc:
        with (
            tc.tile_pool(name="const", bufs=1) as cp,
            tc.tile_pool(name="pep", bufs=pe_bufs) as pep,
            tc.tile_pool(name="wtp", bufs=wt_bufs) as wtp,
            tc.tile_pool(name="accp", bufs=acc_bufs) as accp,
            tc.tile_pool(name="rp", bufs=2) as rp,
            tc.tile_pool(name="outp", bufs=2) as outp,
            tc.tile_pool(name="pp", bufs=1, space="PSUM") as pp,
        ):
            # ---- packed const loads on scalar (HW DGE, runs before relus) --
            nt = nt_all
            fpk = cp.tile([128, 128 + 2 * nkc2], FP8)
            nc.scalar.dma_start(fpk[:], d["fpk"][:])
            cpk = cp.tile([128, 25 * nt], F32)
            nc.scalar.dma_start(cpk[:], d["cpk"][:])
            bias_sb = cp.tile([1, m_own], F32)
            nc.scalar.dma_start(bias_sb[:], d["bias"][:])
            if n_tail:
                tbf = cp.tile([128, n_tail, 4, OC], BF16)
                nc.scalar.dma_start(tbf[:], d["tbf"][:])
                ut = cp.tile([128, OC], F32)
            cind = fpk[:, 0:128]
            xg = fpk[:, 128 : 128 + 2 * nkc2].rearrange(
                "p (r k) -> p r k", r=2, k=nkc2)
            x_all = cpk[:, 0:nt]
            w1 = cpk[:, nt : 10 * nt].rearrange(
                "p (t a b) -> p t a b", t=nt, a=3, b=3)
            b1 = cpk[:, 10 * nt : 13 * nt].rearrange(
                "p (t a) -> p t a", t=nt, a=3)
            w2 = cpk[:, 13 * nt : 22 * nt].rearrange(
                "p (t a b) -> p t a b", t=nt, a=3, b=3)
            b2 = cpk[:, 22 * nt : 25 * nt].rearrange(
                "p (t a) -> p t a", t=nt, a=3)

            # ---- streaming DMA issues, all up-front ------------------------
            # Split the 1MB expert super-tiles across TWO dma queues (sync
            # evens, gpsimd odds): one hwdge queue tops out ~250 GB/s, two
            # sustain the ~400 GB/s the stream needs.  Each queue carries
            # ONLY uniform-size pe tiles, so the scheduler's shortest-job
            # greedy degenerates to consumption order.  wt pair-tiles ride
            # the scalar queue behind the consts (bandwidth-light), and all
            # output stores go to scalar too, where late readiness cannot
            # block the expert stream.
            pets = [pep.tile([128, ns, 2, 2, O_BLK], FP8, tag="pe",
                             name=f"pet{b}")
                    for b in range(NB)]
            wtts = [wtp.tile([128, 2, 2, m_own], FP8, tag="wt",
                             name=f"wtt{j}")
                    for j in range(npair)]
            for b in range(0, NB, 2):
                nc.sync.dma_start(pets[b][:], d["pe"][b : b + 1])
            for b in range(1, NB, 2):
                nc.gpsimd.dma_start(pets[b][:], d["pe"][b : b + 1])
            for j in range(npair):
                nc.scalar.dma_start(wtts[j][:], d["wt"][j : j + 1])

            # ---- reduce selector stationaries (device-built) --------------
            # sel[:, b, r, j] = 0.125 iff j == b : directs block b's 2-row
            # relu-acc contraction into row b of the u psum bank.
            sel = cp.tile([128, NB, nrow, NB], FP8)
            nc.vector.memset(sel[:], 0.0)
            for b in range(NB):
                nc.vector.memset(sel[:, b, :, b : b + 1], 1.0 / H2S)

            # ---- h2 for all experts (slab groups + tail groups) -----------
            h2 = small_mlp(cp, x_all, w1, b1, w2, b2, nt_all, "v")

            # diagonal stationaries: Sa = (8I)*h2_0 | (8I)*h2_1,
            # Sb = (8I)*h2_2 | 8I   (bias row coefficient)
            stat = []
            for s in range(ns):
                Sa = cp.tile([128, 2, 128], FP8, tag=f"Sa{s}")
                Sb = cp.tile([128, 2, 128], FP8, tag=f"Sb{s}")
                for c in (0, 1):
                    nc.vector.tensor_scalar(
                        Sa[:, c, :], cind[:], h2[:, s, c : c + 1], None,
                        op0=ALU.mult,
                    )
                nc.vector.tensor_scalar(
                    Sb[:, 0, :], cind[:], h2[:, s, 2:3], None, op0=ALU.mult,
                )
                nc.vector.tensor_copy(Sb[:, 1, :], cind[:])
                stat.append((Sa, Sb))

            # ---- persistent psum tiles ------------------------------------
            u_all = pp.tile([NB, O_BLK], F32, tag="uall")
            d_psum = pp.tile([1, m_own], F32, tag="dpsum")

            # ---- main streamed loop ---------------------------------------
            pend = None            # (block, acc) awaiting its reduce matmul
            for b in range(NB):
                pet = pets[b]
                # ---------- PE path: 2 DoubleRow matmuls per slab -----------
                stks = []
                for s in range(ns):
                    stk = pp.tile([128, O_BLK], F32, tag=f"stk{s}")
                    Sa, Sb = stat[s]
                    nc.tensor.matmul(
                        stk[:], Sa[:], pet[:, s, 0, :, :],
                        start=True, stop=False, perf_mode=PM.DoubleRow,
                    )
                    nc.tensor.matmul(
                        stk[:], Sb[:], pet[:, s, 1, :, :],
                        start=False, stop=True, perf_mode=PM.DoubleRow,
                    )
                    stks.append(stk)

                # ---------- dense matvec pair-tile (DoubleRow) --------------
                if b < npair:
                    wtt = wtts[b]
                    for q in (0, 1):
                        kc = 2 * b + q
                        for mb in range(m_own // 512):
                            lo, hi = mb * 512, (mb + 1) * 512
                            nc.tensor.matmul(
                                d_psum[0:1, lo:hi],
                                xg[:, :, kc : kc + 1],
                                wtt[:, q, :, lo:hi],
                                start=(kc == 0), stop=(kc == nkc2 - 1),
                                perf_mode=PM.DoubleRow,
                            )
                    if b == npair - 1:
                        dense_sb = outp.tile([1, m_own], F32, tag="dense_sb")
                        nc.vector.scalar_tensor_tensor(
                            dense_sb[:], d_psum[:], 1.0 / WT_SCALE, bias_sb[:],
                            op0=ALU.mult, op1=ALU.add,
                        )
                        nc.scalar.dma_start(d["dense_out"][:], dense_sb[:])

                # ---------- deferred reduce for the previous block ----------
                if pend is not None:
                    pb, pacc = pend
                    nc.tensor.matmul(
                        u_all[:], sel[:, pb, :, :], pacc[:],
                        start=(pb == 0), stop=(pb == NB - 1),
                        perf_mode=PM.DoubleRow,
                    )

                # ---------- relu-accumulate chains (Scalar + DVE) -----------
                acc = accp.tile([128, nrow, O_BLK], FP8, tag="acc")
                for r in range(nrow):
                    nc.scalar.activation(acc[:, r, :], stks[2 * r][:], AF.Relu)
                    nc.vector.scalar_tensor_tensor(
                        acc[:, r, :], stks[2 * r + 1][:], 0.0, acc[:, r, :],
                        op0=ALU.max, op1=ALU.add,
                    )
                pend = (b, acc)

                # ---------- tail experts, once, early ----------
                if n_tail and b == tail_blk:
                    for e in range(n_tail):
                        tacc = rp.tile([128, OC], BF16, tag=f"tacc{e}")
                        nc.vector.scalar_tensor_tensor(
                            tacc[:], tbf[:, e, 0, :], h2[:, ns + e, 0:1],
                            tbf[:, e, 3, :], op0=ALU.mult, op1=ALU.add,
                        )
                        for i in (1, 2):
                            nc.vector.scalar_tensor_tensor(
                                tacc[:], tbf[:, e, i, :], h2[:, ns + e, i : i + 1],
                                tacc[:], op0=ALU.mult, op1=ALU.add,
                            )
                        if e == 0:
                            nc.scalar.activation(ut[:], tacc[:], AF.Relu)
                        else:
                            rt = rp.tile([128, OC], F32, tag="rt")
                            nc.scalar.activation(rt[:], tacc[:], AF.Relu)
                            nc.vector.tensor_tensor(ut[:], ut[:], rt[:], op=ALU.add)
                    nc.scalar.dma_start(d["ut_out"][:], ut[:])

            # ---------- final reduce + single u store ----------
            pb, pacc = pend
            nc.tensor.matmul(
                u_all[:], sel[:, pb, :, :], pacc[:],
                start=(pb == 0), stop=True, perf_mode=PM.DoubleRow,
            )
            u_sb = outp.tile([NB, O_BLK], F32, tag="u_sb")
            nc.vector.tensor_copy(u_sb[:], u_all[:])
            nc.sync.dma_start(d["u_out"][:], u_sb[:])

    nc.compile()
    return nc, d


def route(inputs):
    """Host-side routing: active experts + active dense columns."""
    x = np.asarray(inputs["x"], dtype=np.float32)
    sat = np.asarray(inputs["saturated"]).astype(bool)
    act = np.nonzero(sat & (x != 0))[0]
    dcols = np.nonzero(~sat)[0]
    per = -(-len(act) // N_CORES)            # ceil
    nslab = per // 128                       # full 128-expert slabs
    if nslab % 2:                            # DR reduce pairs slabs
        nslab -= 1
    n_tail = per - 128 * nslab
    nkc2 = -(-len(dcols) // 256)
    if nkc2 % 2:
        nkc2 += 1                            # dense pair-tiles need even kc
    return act, dcols, per, 0, nslab, n_tail, nkc2


def make_in_maps(inputs, act, dcols, per, nsub, nslab, n_tail, nkc2):
    x = np.asarray(inputs["x"], dtype=np.float32)
    weight = np.asarray(inputs["weight"], dtype=np.float32)
    bias = np.asarray(inputs["bias"], dtype=np.float32)
    W1 = np.asarray(inputs["W1"], dtype=np.float32)
    b1 = np.asarray(inputs["b1"], dtype=np.float32)
    W2 = np.asarray(inputs["W2"], dtype=np.float32)
    b2 = np.asarray(inputs["b2"], dtype=np.float32)
    W3 = np.asarray(inputs["W3"], dtype=np.float32)
    b3 = np.asarray(inputs["b3"], dtype=np.float32)

    ns = nslab
    m_own = SIZE_OUT // N_CORES
    NB = SIZE_OUT // O_BLK
    npair = nkc2 // 2
    n_slab = 128 * ns
    Dp = nkc2 * 256

    W38 = W3.astype(NP_FP8)                  # [N, O, 3]
    b38 = b3.astype(NP_FP8)                  # [N, O]

    xg_full = np.zeros(Dp, dtype=np.float32)
    xg_full[: len(dcols)] = x[dcols]
    # DoubleRow pairs: partition p of chunk kc holds rows kc*256+2p, +1
    xg = np.ascontiguousarray(
        xg_full.reshape(nkc2, 128, 2).transpose(1, 2, 0)
    ).astype(NP_FP8)

    cind = (H2S * np.eye(128, dtype=np.float32)).astype(NP_FP8)

    in_maps = []
    for i in range(N_CORES):
        ids = act[i * per : (i + 1) * per]
        n_live = len(ids)
        if n_live < per:
            ids = np.concatenate([ids, np.zeros(per - n_live, dtype=ids.dtype)])
        gids = ids[:n_slab]
        tids = ids[n_slab:]

        # ---- contiguous per-o-block expert super-tiles -------------------
        G = np.empty((n_slab, SIZE_OUT, 4), dtype=NP_FP8)
        G[:, :, 0:3] = W38[gids]
        G[:, :, 3] = b38[gids]
        live = min(max(n_live, 0), n_slab)
        if live < n_slab:
            G[live:] = 0
        pe = np.ascontiguousarray(
            G.reshape(ns, 128, NB, O_BLK, 4).transpose(2, 1, 0, 4, 3)
        ).reshape(NB, 128, ns, 2, 2, O_BLK)

        # ---- merged small-MLP consts (slab groups + tail groups) ---------
        def grp(a, shp):
            main = a[gids].reshape((ns, 128) + shp).transpose(
                (1, 0) + tuple(range(2, 2 + len(shp))))
            if n_tail:
                tailb = np.broadcast_to(a[tids], (128, n_tail) + shp)
                main = np.concatenate([main, tailb], axis=1)
            return main.reshape(128, -1)

        nt = ns + n_tail
        cpkarr = np.ascontiguousarray(np.concatenate(
            [grp(x, ()), grp(W1, (3, 3)), grp(b1, (3,)),
             grp(W2, (3, 3)), grp(b2, (3,))], axis=1, dtype=np.float32))
        fpkarr = np.empty((128, 128 + 2 * nkc2), dtype=NP_FP8)
        fpkarr[:, 0:128] = cind
        fpkarr[:, 128:] = xg.reshape(128, 2 * nkc2)

        m = {
            "pe": pe,
            "fpk": fpkarr,
            "cpk": cpkarr,
            "bias": bias[i * m_own : (i + 1) * m_own].reshape(1, m_own),
        }

        slm = slice(i * m_own, (i + 1) * m_own)
        wtg = np.zeros((Dp, m_own), dtype=np.float32)
        wtg[: len(dcols)] = weight[slm][:, dcols].T * WT_SCALE
        m["wt"] = np.ascontiguousarray(
            wtg.astype(NP_FP8).reshape(npair, 2, 128, 2, m_own)
            .transpose(0, 2, 1, 3, 4)
        )

        if n_tail:
            nt_live = max(0, min(n_tail, n_live - n_slab))
            w3tt = np.ascontiguousarray(
                W3[tids]
                .transpose(0, 2, 1)
                .reshape(n_tail, 3, OC, 128)
                .transpose(3, 0, 1, 2)
            ).astype(NP_BF16)
            b3tt = np.ascontiguousarray(
                b3[tids].reshape(n_tail, OC, 128).transpose(2, 0, 1)
            ).astype(NP_BF16)
            if nt_live < n_tail:
                w3tt[:, nt_live:] = 0
                b3tt[:, nt_live:] = 0
            tbf = np.empty((128, n_tail, 4, OC), dtype=NP_BF16)
            tbf[:, :, 0:3, :] = w3tt
            tbf[:, :, 3, :] = b3tt
            m["tbf"] = tbf
        in_maps.append(m)
    return in_maps


def combine_outputs(results, names, n_tail):
    u = np.zeros(SIZE_OUT, dtype=np.float64)
    dense = []
    for res in results:
        u += res[names["u_out"].name].reshape(-1).astype(np.float64)
        if n_tail:
            ut = res[names["ut_out"].name].astype(np.float64)  # [128, OC]
            u += ut.T.reshape(-1)                              # o = c*128 + p
        dense.append(res[names["dense_out"].name].reshape(-1))
    out = np.concatenate(dense).astype(np.float64) + u
    return out.astype(np.float32)


_CACHE = {}
CONFIG = {}


def _get_program(nsub, nslab, n_tail, nkc2):
    key = (nsub, nslab, n_tail, nkc2, tuple(sorted(CONFIG.items())))
    if key not in _CACHE:
        _CACHE[key] = build_program(nslab, n_tail, nkc2, **CONFIG)
    return _CACHE[key]


def kernel(**inputs):
    act, dcols, per, nsub, nslab, n_tail, nkc2 = route(inputs)
    nc, names = _get_program(nsub, nslab, n_tail, nkc2)
    in_maps = make_in_maps(inputs, act, dcols, per, nsub, nslab, n_tail, nkc2)
    keyed = [{names[k].name: v for k, v in m.items()} for m in in_maps]
    res = run_bass_kernel_spmd(nc, keyed, core_ids=list(range(N_CORES)))
    return combine_outputs(res.results, names, n_tail)


# revision 17
# speedup vs baseline: 1.1083x; 1.1083x over previous
"""BuddingLayer Trainium2 kernel (8-core, routed + fp8, contiguous-stream).

Reference computation (N = size_in = 8192, O = size_out = 8192):
    mask  = (x * saturated) != 0                   # ~half the neurons
    h2    = tiny per-neuron MLP(x)                              [N,3]
    h3    = relu(sum_i W3[n,o,i] * h2[n,i] + b3[n,o])           [N,O]
    u[o]  = sum_n mask[n] * h3[n,o]
    out   = weight @ (x * ~saturated) + bias + u

Host-side routing gathers the ~4112 active experts (mask=1) and ~4080
active dense columns; all big streams are fp8e4 (tolerance 2e-2).
Per-core stream ~21 MB -> ~60 us DMA floor at 358 GB/s.

v2 design (from 113.5us baseline profile):
  * The baseline spent ~130us of aggregate issue-engine time on ~200
    small strided DMAs (512B packets) and ~25us of Tensor time on bf16
    ones-reduce matmuls.  Both are restructured away:
  * Expert stream is pre-tiled on host into per-o-block CONTIGUOUS
    super-tiles pe[b] = [128, ns, 2(pair), 2(row), 512] fp8 -> ONE
    1 MB DMA per block (16 total), 8 KB/partition contiguous.
  * All ns=4 slabs go through the PE DoubleRow path (2 matmuls/slab
    into stk_s).  relu is a 2-op chain per psum tile pair: Scalar
    ACTIVATE(relu) writes row r of a [128,2,512] fp8 acc tile for slab
    2r, DVE scalar_tensor_tensor max-add accumulates slab 2r+1.
  * One DoubleRow reduce matmul per block contracts acc against a
    0.125-valued block-column selector -> row b of a single [16,512]
    psum bank accumulated across all 16 blocks; one copy + one store
    at the end.  (0.125 = 1/H2S undoes the stationary's 8x scale.)
  * Dense matvec: wt pre-tiled into 8 contiguous pair-tiles
    [128, 2(kc), 2(row), 1024] fp8, DoubleRow with xg stationary.
  * Small MLP consts for the 4 slab groups + 2 tail experts are merged
    (nt=6) into single per-field DMAs.
"""

import sys

import numpy as np

_TRN = "/opt/trn_rl_repo"
if _TRN not in sys.path:
    sys.path.insert(0, _TRN)

import ml_dtypes

import concourse.bacc as bacc
import concourse.mybir as mybir
from concourse import tile
from concourse.bass_utils import run_bass_kernel_spmd

F32 = mybir.dt.float32
BF16 = mybir.dt.bfloat16
FP8 = mybir.dt.float8e4
AF = mybir.ActivationFunctionType
ALU = mybir.AluOpType
AX = mybir.AxisListType
PM = mybir.MatmulPerfMode

NP_BF16 = ml_dtypes.bfloat16
NP_FP8 = ml_dtypes.float8_e4m3

N_CORES = 8
SIZE_IN = 8192
SIZE_OUT = 8192
OC = SIZE_OUT // 128          # o-chunks for the tail layout
O_BLK = 512                   # o-block (one psum bank of f32)
WT_SCALE = 1024.0             # dense weights are subnormal in fp8; prescale
H2S = 8.0                     # h2 scale in the PE stationary (undone in reduce)


def build_program(
    ns,                        # 128-expert PE slabs per core (must be even)
    n_tail,                    # leftover experts per core (o-transposed path)
    nkc2,                      # 256-row DoubleRow chunks for dense (even)
    size_out=SIZE_OUT,
    n_cores=N_CORES,
    pe_bufs=16,
    wt_bufs=8,
    acc_bufs=3,
    tail_blk=10,
    enable_asserts=False,
):
    assert ns % 2 == 0 and ns >= 2
    assert nkc2 % 2 == 0
    m_own = size_out // n_cores
    NB = size_out // O_BLK
    npair = nkc2 // 2
    nrow = ns // 2
    nt_all = ns + n_tail       # merged small-MLP groups (slab + tail)

    nc = bacc.Bacc(
        "TRN2",
        target_bir_lowering=False,
        debug=False,
        enable_asserts=enable_asserts,
        num_devices=n_cores,
    )

    d = {}
    d["pe"] = nc.dram_tensor("pe", [NB, 128, ns, 2, 2, O_BLK], FP8,
                             kind="ExternalInput")
    d["wt"] = nc.dram_tensor("wt", [npair, 128, 2, 2, m_own], FP8,
                             kind="ExternalInput")
    # packed consts: fpk = cind | xg (fp8), cpk = x | w1 | b1 | w2 | b2 (f32)
    d["fpk"] = nc.dram_tensor("fpk", [128, 128 + 2 * nkc2], FP8,
                              kind="ExternalInput")
    d["cpk"] = nc.dram_tensor("cpk", [128, 25 * nt_all], F32,
                              kind="ExternalInput")
    d["bias"] = nc.dram_tensor("bias", [1, m_own], F32, kind="ExternalInput")
    if n_tail:
        d["tbf"] = nc.dram_tensor("tbf", [128, n_tail, 4, OC], BF16,
                                  kind="ExternalInput")
        d["ut_out"] = nc.dram_tensor("ut_out", [128, OC], F32,
                                     kind="ExternalOutput")
    d["u_out"] = nc.dram_tensor("u_out", [NB, O_BLK], F32, kind="ExternalOutput")
    d["dense_out"] = nc.dram_tensor("dense_out", [1, m_own], F32,
                                    kind="ExternalOutput")

    def small_mlp(cp, x_sb, w1, b1, w2, b2, nt, pfx):
        h0 = cp.tile([128, nt], F32, tag=f"{pfx}h0")
        nc.vector.tensor_scalar_mul(h0[:], x_sb[:], 1.0 / 3.0)
        rs1 = cp.tile([128, nt, 3], F32, tag=f"{pfx}rs1")
        nc.vector.tensor_reduce(rs1[:], w1[:], axis=AX.X, op=ALU.add)
        h1 = cp.tile([128, nt, 3], F32, tag=f"{pfx}h1")
        for t in range(nt):
            nc.vector.scalar_tensor_tensor(
                h1[:, t, :], rs1[:, t, :], h0[:, t : t + 1], b1[:, t, :],
                op0=ALU.mult, op1=ALU.add,
            )
        nc.vector.tensor_scalar_max(h1[:], h1[:], 0.0)
        h2 = cp.tile([128, nt, 3], F32, tag=f"{pfx}h2")
        for t in range(nt):
            nc.vector.scalar_tensor_tensor(
                h2[:, t, :], w2[:, t, :, 0], h1[:, t, 0:1], b2[:, t, :],
                op0=ALU.mult, op1=ALU.add,
            )
            for i in (1, 2):
                nc.vector.scalar_tensor_tensor(
                    h2[:, t, :], w2[:, t, :, i], h1[:, t, i : i + 1], h2[:, t, :],
                    op0=ALU.mult, op1=ALU.add,
                )
        nc.vector.tensor_scalar_max(h2[:], h2[:], 0.0)
        return h2

    with tile.TileContext(nc) as tc:
        with (
            tc.tile_pool(name="const", bufs=1) as cp,
            tc.tile_pool(name="pep", bufs=pe_bufs) as pep,
            tc.tile_pool(name="wtp", bufs=wt_bufs) as wtp,
            tc.tile_pool(name="accp", bufs=acc_bufs) as accp,
            tc.tile_pool(name="rp", bufs=2) as rp,
            tc.tile_pool(name="outp", bufs=2) as outp,
            tc.tile_pool(name="pp", bufs=1, space="PSUM") as pp,
        ):
            # ---- packed const loads on scalar (HW DGE, runs before relus) --
            nt = nt_all
            fpk = cp.tile([128, 128 + 2 * nkc2], FP8)
            nc.scalar.dma_start(fpk[:], d["fpk"][:])
            cpk = cp.tile([128, 25 * nt], F32)
            nc.scalar.dma_start(cpk[:], d["cpk"][:])
            bias_sb = cp.tile([1, m_own], F32)
            nc.scalar.dma_start(bias_sb[:], d["bias"][:])
            if n_tail:
                tbf = cp.tile([128, n_tail, 4, OC], BF16)
                nc.scalar.dma_start(tbf[:], d["tbf"][:])
                ut = cp.tile([128, OC], F32)
            cind = fpk[:, 0:128]
            xg = fpk[:, 128 : 128 + 2 * nkc2].rearrange(
                "p (r k) -> p r k", r=2, k=nkc2)
            x_all = cpk[:, 0:nt]
            w1 = cpk[:, nt : 10 * nt].rearrange(
                "p (t a b) -> p t a b", t=nt, a=3, b=3)
            b1 = cpk[:, 10 * nt : 13 * nt].rearrange(
                "p (t a) -> p t a", t=nt, a=3)
            w2 = cpk[:, 13 * nt : 22 * nt].rearrange(
                "p (t a b) -> p t a b", t=nt, a=3, b=3)
            b2 = cpk[:, 22 * nt : 25 * nt].rearrange(
                "p (t a) -> p t a", t=nt, a=3)

            # ---- streaming DMA issues, all up-front ------------------------
            # Split the 1MB expert super-tiles across TWO dma queues (sync
            # evens, gpsimd odds): one hwdge queue tops out ~250 GB/s, two
            # sustain the ~400 GB/s the stream needs.  Each queue carries
            # ONLY uniform-size pe tiles, so the scheduler's shortest-job
            # greedy degenerates to consumption order.  wt pair-tiles ride
            # the scalar queue behind the consts (bandwidth-light), and all
            # output stores go to scalar too, where late readiness cannot
            # block the expert stream.
            pets = [pep.tile([128, ns, 2, 2, O_BLK], FP8, tag="pe",
                             name=f"pet{b}")
                    for b in range(NB)]
            wtts = [wtp.tile([128, 2, 2, m_own], FP8, tag="wt",
                             name=f"wtt{j}")
                    for j in range(npair)]
            for b in range(0, NB, 2):
                nc.sync.dma_start(pets[b][:], d["pe"][b : b + 1])
            for b in range(1, NB, 2):
                nc.scalar.dma_start(pets[b][:], d["pe"][b : b + 1])
            for j in range(npair):
                nc.gpsimd.dma_start(wtts[j][:], d["wt"][j : j + 1])

            # ---- reduce selector stationaries (device-built) --------------
            # sel[:, b, r, j] = 0.125 iff j == b : directs block b's 2-row
            # relu-acc contraction into row b of the u psum bank.
            sel = cp.tile([128, NB, nrow, NB], FP8)
            nc.vector.memset(sel[:], 0.0)
            for b in range(NB):
                nc.vector.memset(sel[:, b, :, b : b + 1], 1.0 / H2S)

            # ---- h2 for all experts (slab groups + tail groups) -----------
            h2 = small_mlp(cp, x_all, w1, b1, w2, b2, nt_all, "v")

            # diagonal stationaries: Sa = (8I)*h2_0 | (8I)*h2_1,
            # Sb = (8I)*h2_2 | 8I   (bias row coefficient)
            stat = []
            for s in range(ns):
                Sa = cp.tile([128, 2, 128], FP8, tag=f"Sa{s}")
                Sb = cp.tile([128, 2, 128], FP8, tag=f"Sb{s}")
                for c in (0, 1):
                    nc.vector.tensor_scalar(
                        Sa[:, c, :], cind[:], h2[:, s, c : c + 1], None,
                        op0=ALU.mult,
                    )
                nc.vector.tensor_scalar(
                    Sb[:, 0, :], cind[:], h2[:, s, 2:3], None, op0=ALU.mult,
                )
                nc.vector.tensor_copy(Sb[:, 1, :], cind[:])
                stat.append((Sa, Sb))

            # ---- persistent psum tiles ------------------------------------
            u_all = pp.tile([NB, O_BLK], F32, tag="uall")
            d_psum = pp.tile([1, m_own], F32, tag="dpsum")

            # ---- main streamed loop ---------------------------------------
            pend = None            # (block, acc) awaiting its reduce matmul
            for b in range(NB):
                pet = pets[b]
                # ---------- PE path: 2 DoubleRow matmuls per slab -----------
                stks = []
                for s in range(ns):
                    stk = pp.tile([128, O_BLK], F32, tag=f"stk{s}")
                    Sa, Sb = stat[s]
                    nc.tensor.matmul(
                        stk[:], Sa[:], pet[:, s, 0, :, :],
                        start=True, stop=False, perf_mode=PM.DoubleRow,
                    )
                    nc.tensor.matmul(
                        stk[:], Sb[:], pet[:, s, 1, :, :],
                        start=False, stop=True, perf_mode=PM.DoubleRow,
                    )
                    stks.append(stk)

                # ---------- dense matvec pair-tile (DoubleRow) --------------
                # pair j consumed at block 2j: halves the wt need-rate so
                # the software-DGE gpsimd queue (~170 GB/s) keeps up
                if b % 2 == 0 and b // 2 < npair:
                    wtt = wtts[b // 2]
                    for q in (0, 1):
                        kc = b + q
                        for mb in range(m_own // 512):
                            lo, hi = mb * 512, (mb + 1) * 512
                            nc.tensor.matmul(
                                d_psum[0:1, lo:hi],
                                xg[:, :, kc : kc + 1],
                                wtt[:, q, :, lo:hi],
                                start=(kc == 0), stop=(kc == nkc2 - 1),
                                perf_mode=PM.DoubleRow,
                            )
                    if b == 2 * (npair - 1):
                        dense_sb = outp.tile([1, m_own], F32, tag="dense_sb")
                        nc.vector.scalar_tensor_tensor(
                            dense_sb[:], d_psum[:], 1.0 / WT_SCALE, bias_sb[:],
                            op0=ALU.mult, op1=ALU.add,
                        )
                        nc.scalar.dma_start(d["dense_out"][:], dense_sb[:])

                # ---------- deferred reduce for the previous block ----------
                if pend is not None:
                    pb, pacc = pend
                    nc.tensor.matmul(
                        u_all[:], sel[:, pb, :, :], pacc[:],
                        start=(pb == 0), stop=(pb == NB - 1),
                        perf_mode=PM.DoubleRow,
                    )

                # ---------- relu-accumulate chains (Scalar + DVE) -----------
                acc = accp.tile([128, nrow, O_BLK], FP8, tag="acc")
                for r in range(nrow):
                    nc.scalar.activation(acc[:, r, :], stks[2 * r][:], AF.Relu)
                    nc.vector.scalar_tensor_tensor(
                        acc[:, r, :], stks[2 * r + 1][:], 0.0, acc[:, r, :],
                        op0=ALU.max, op1=ALU.add,
                    )
                pend = (b, acc)

                # ---------- tail experts, once, early ----------
                if n_tail and b == tail_blk:
                    for e in range(n_tail):
                        tacc = rp.tile([128, OC], BF16, tag=f"tacc{e}")
                        nc.vector.scalar_tensor_tensor(
                            tacc[:], tbf[:, e, 0, :], h2[:, ns + e, 0:1],
                            tbf[:, e, 3, :], op0=ALU.mult, op1=ALU.add,
                        )
                        for i in (1, 2):
                            nc.vector.scalar_tensor_tensor(
                                tacc[:], tbf[:, e, i, :], h2[:, ns + e, i : i + 1],
                                tacc[:], op0=ALU.mult, op1=ALU.add,
                            )
                        if e == 0:
                            nc.scalar.activation(ut[:], tacc[:], AF.Relu)
                        else:
                            rt = rp.tile([128, OC], F32, tag="rt")
                            nc.scalar.activation(rt[:], tacc[:], AF.Relu)
                            nc.vector.tensor_tensor(ut[:], ut[:], rt[:], op=ALU.add)
                    nc.scalar.dma_start(d["ut_out"][:], ut[:])

            # ---------- final reduce + single u store ----------
            pb, pacc = pend
            nc.tensor.matmul(
                u_all[:], sel[:, pb, :, :], pacc[:],
                start=(pb == 0), stop=True, perf_mode=PM.DoubleRow,
            )
            u_sb = outp.tile([NB, O_BLK], F32, tag="u_sb")
            nc.vector.tensor_copy(u_sb[:], u_all[:])
            nc.sync.dma_start(d["u_out"][:], u_sb[:])

    nc.compile()
    return nc, d


def route(inputs):
    """Host-side routing: active experts + active dense columns."""
    x = np.asarray(inputs["x"], dtype=np.float32)
    sat = np.asarray(inputs["saturated"]).astype(bool)
    act = np.nonzero(sat & (x != 0))[0]
    dcols = np.nonzero(~sat)[0]
    per = -(-len(act) // N_CORES)            # ceil
    nslab = per // 128                       # full 128-expert slabs
    if nslab % 2:                            # DR reduce pairs slabs
        nslab -= 1
    n_tail = per - 128 * nslab
    nkc2 = -(-len(dcols) // 256)
    if nkc2 % 2:
        nkc2 += 1                            # dense pair-tiles need even kc
    return act, dcols, per, 0, nslab, n_tail, nkc2


def make_in_maps(inputs, act, dcols, per, nsub, nslab, n_tail, nkc2):
    x = np.asarray(inputs["x"], dtype=np.float32)
    weight = np.asarray(inputs["weight"], dtype=np.float32)
    bias = np.asarray(inputs["bias"], dtype=np.float32)
    W1 = np.asarray(inputs["W1"], dtype=np.float32)
    b1 = np.asarray(inputs["b1"], dtype=np.float32)
    W2 = np.asarray(inputs["W2"], dtype=np.float32)
    b2 = np.asarray(inputs["b2"], dtype=np.float32)
    W3 = np.asarray(inputs["W3"], dtype=np.float32)
    b3 = np.asarray(inputs["b3"], dtype=np.float32)

    ns = nslab
    m_own = SIZE_OUT // N_CORES
    NB = SIZE_OUT // O_BLK
    npair = nkc2 // 2
    n_slab = 128 * ns
    Dp = nkc2 * 256

    W38 = W3.astype(NP_FP8)                  # [N, O, 3]
    b38 = b3.astype(NP_FP8)                  # [N, O]

    xg_full = np.zeros(Dp, dtype=np.float32)
    xg_full[: len(dcols)] = x[dcols]
    # DoubleRow pairs: partition p of chunk kc holds rows kc*256+2p, +1
    xg = np.ascontiguousarray(
        xg_full.reshape(nkc2, 128, 2).transpose(1, 2, 0)
    ).astype(NP_FP8)

    cind = (H2S * np.eye(128, dtype=np.float32)).astype(NP_FP8)

    in_maps = []
    for i in range(N_CORES):
        ids = act[i * per : (i + 1) * per]
        n_live = len(ids)
        if n_live < per:
            ids = np.concatenate([ids, np.zeros(per - n_live, dtype=ids.dtype)])
        gids = ids[:n_slab]
        tids = ids[n_slab:]

        # ---- contiguous per-o-block expert super-tiles -------------------
        G = np.empty((n_slab, SIZE_OUT, 4), dtype=NP_FP8)
        G[:, :, 0:3] = W38[gids]
        G[:, :, 3] = b38[gids]
        live = min(max(n_live, 0), n_slab)
        if live < n_slab:
            G[live:] = 0
        pe = np.ascontiguousarray(
            G.reshape(ns, 128, NB, O_BLK, 4).transpose(2, 1, 0, 4, 3)
        ).reshape(NB, 128, ns, 2, 2, O_BLK)

        # ---- merged small-MLP consts (slab groups + tail groups) ---------
        def grp(a, shp):
            main = a[gids].reshape((ns, 128) + shp).transpose(
                (1, 0) + tuple(range(2, 2 + len(shp))))
            if n_tail:
                tailb = np.broadcast_to(a[tids], (128, n_tail) + shp)
                main = np.concatenate([main, tailb], axis=1)
            return main.reshape(128, -1)

        nt = ns + n_tail
        cpkarr = np.ascontiguousarray(np.concatenate(
            [grp(x, ()), grp(W1, (3, 3)), grp(b1, (3,)),
             grp(W2, (3, 3)), grp(b2, (3,))], axis=1, dtype=np.float32))
        fpkarr = np.empty((128, 128 + 2 * nkc2), dtype=NP_FP8)
        fpkarr[:, 0:128] = cind
        fpkarr[:, 128:] = xg.reshape(128, 2 * nkc2)

        m = {
            "pe": pe,
            "fpk": fpkarr,
            "cpk": cpkarr,
            "bias": bias[i * m_own : (i + 1) * m_own].reshape(1, m_own),
        }

        slm = slice(i * m_own, (i + 1) * m_own)
        wtg = np.zeros((Dp, m_own), dtype=np.float32)
        wtg[: len(dcols)] = weight[slm][:, dcols].T * WT_SCALE
        m["wt"] = np.ascontiguousarray(
            wtg.astype(NP_FP8).reshape(npair, 2, 128, 2, m_own)
            .transpose(0, 2, 1, 3, 4)
        )

        if n_tail:
            nt_live = max(0, min(n_tail, n_live - n_slab))
            w3tt = np.ascontiguousarray(
                W3[tids]
                .transpose(0, 2, 1)
                .reshape(n_tail, 3, OC, 128)
                .transpose(3, 0, 1, 2)
            ).astype(NP_BF16)
            b3tt = np.ascontiguousarray(
                b3[tids].reshape(n_tail, OC, 128).transpose(2, 0, 1)
            ).astype(NP_BF16)
            if nt_live < n_tail:
                w3tt[:, nt_live:] = 0
                b3tt[:, nt_live:] = 0
            tbf = np.empty((128, n_tail, 4, OC), dtype=NP_BF16)
            tbf[:, :, 0:3, :] = w3tt
            tbf[:, :, 3, :] = b3tt
            m["tbf"] = tbf
        in_maps.append(m)
    return in_maps


def combine_outputs(results, names, n_tail):
    u = np.zeros(SIZE_OUT, dtype=np.float64)
    dense = []
    for res in results:
        u += res[names["u_out"].name].reshape(-1).astype(np.float64)
        if n_tail:
            ut = res[names["ut_out"].name].astype(np.float64)  # [128, OC]
            u += ut.T.reshape(-1)                              # o = c*128 + p
        dense.append(res[names["dense_out"].name].reshape(-1))
    out = np.concatenate(dense).astype(np.float64) + u
    return out.astype(np.float32)


_CACHE = {}
CONFIG = {}


def _get_program(nsub, nslab, n_tail, nkc2):
    key = (nsub, nslab, n_tail, nkc2, tuple(sorted(CONFIG.items())))
    if key not in _CACHE:
        _CACHE[key] = build_program(nslab, n_tail, nkc2, **CONFIG)
    return _CACHE[key]


def kernel(**inputs):
    act, dcols, per, nsub, nslab, n_tail, nkc2 = route(inputs)
    nc, names = _get_program(nsub, nslab, n_tail, nkc2)
    in_maps = make_in_maps(inputs, act, dcols, per, nsub, nslab, n_tail, nkc2)
    keyed = [{names[k].name: v for k, v in m.items()} for m in in_maps]
    res = run_bass_kernel_spmd(nc, keyed, core_ids=list(range(N_CORES)))
    return combine_outputs(res.results, names, n_tail)
